# revision 27
# baseline (speedup 1.0000x reference)
"""Trainium2 Bass kernel for nn_CorrectorEGNN (B=128 graphs, N=64 nodes, H=128, L=4).

Data-parallel over graphs (16 graphs/core x 8 cores). Dense 64x64 edge grid in
dst-major order: e = j*64 + i (src=i, dst=j). The i==j diagonal is subtracted
only where it matters (msg) — it cancels algebraically in the pos update.

Edge-MLP first layer restructured as
  e_in @ W1 = A''[src] + B''[dst] - 2*wrow (x) gram_row
so stage 1 is one K=128 bf16 matmul against a constant 0/1 selection matrix S
plus rank-1 matmuls against the flattened Gram row. The Gram matrix is computed
in fp32 and split hi/lo into two bf16 rows so the d2 = gi+gj-2gij cancellation
happens at ~fp32 precision inside the PSUM accumulation. All edge-path matmuls
run in bf16 (1 cycle/row); positions stay fp32.

Scheduling: pos update for layer l is deferred to the start of layer l+1
(cw matrix parked in DRAM), per-graph pre-work is emitted one graph ahead of
the edge-chunk pipeline, and edge matmuls are stage-batched so the in-order
PE/DVE queues never block on the long chunk dependency chain. cw2 runs as
4 column-tiled concurrent matmuls (output partitions 0/32/64/96) so the
PSUM->SBUF copy of the cw row moves 4 edges per DVE cycle instead of 1.
msg is the per-dst sum of m: one GpSimd halving fold + a DVE inner reduce.
"""

import sys

sys.path.insert(0, "/opt/trn_rl_repo")

import numpy as np

N = 64
C = 3
H = 128
L = 4
B = 128
NCORES = 8
GPC = B // NCORES  # graphs per core
E = N * N  # dense edges per graph
CH = 1024  # edge columns per pipeline chunk
NCH = E // CH

_CACHE = {}


def _prep_consts(inputs):
    """Numpy-side packing of weights into DMA-friendly layouts (replicated per core)."""
    f32 = np.float32
    ew1 = np.asarray(inputs["edge_w1"], f32)  # [L, 2H+1, H]
    d = {}
    d["w1a"] = np.concatenate([ew1[l, :H] for l in range(L)], axis=1)  # [128, 512]
    d["w1b"] = np.concatenate([ew1[l, H : 2 * H] for l in range(L)], axis=1)
    wrow = ew1[:, 2 * H]  # [L, 128]
    d["wrow"] = np.concatenate([wrow[l][None, :] for l in range(L)], axis=1)  # [1, 512]
    d["wm2"] = np.concatenate([(-2.0 * wrow[l])[None, :] for l in range(L)], axis=1)
    d["w2"] = np.concatenate([np.asarray(inputs["edge_w2"], f32)[l] for l in range(L)], axis=1)
    d["cw1"] = np.concatenate([np.asarray(inputs["coord_w1"], f32)[l] for l in range(L)], axis=1)
    d["cw2"] = np.concatenate([np.asarray(inputs["coord_w2"], f32)[l] for l in range(L)], axis=1)  # [128, 4]
    nw1 = np.asarray(inputs["node_w1"], f32)
    d["nw1a"] = np.concatenate([nw1[l, :H] for l in range(L)], axis=1)
    d["nw1b"] = np.concatenate([nw1[l, H:] for l in range(L)], axis=1)
    d["nw2"] = np.concatenate([np.asarray(inputs["node_w2"], f32)[l] for l in range(L)], axis=1)
    # biases: [128, 5*L]; column order b1(l), b2(l), cb1(l), nb1(l), nb2(l)
    bias_cols = []
    for nm in ("edge_b1", "edge_b2", "coord_b1", "node_b1", "node_b2"):
        arr = np.asarray(inputs[nm], f32)  # [L, 128]
        for l in range(L):
            bias_cols.append(arr[l][:, None])
    d["biases"] = np.concatenate(bias_cols, axis=1)  # [128, 20]
    d["nerep"] = np.tile(np.asarray(inputs["node_embed"], f32).T, (1, N))  # [128, 64]
    d["ident"] = np.eye(N, dtype=f32)
    os_val = float(np.asarray(inputs["output_scale"], f32)[0])
    msc = np.zeros((N, 2), f32)
    msc[:, 0] = 1.0
    msc[:, 1] = os_val
    d["msc"] = msc
    d["inv64"] = np.full((1, N), 1.0 / N, f32)
    d["ones3"] = np.ones((C, 1), f32)
    d["onesr"] = np.ones((1, N), f32)
    # selection matrix S [128, E], dst-major edges e=(j,i):
    # rows 0-63 pick src i (I_64 tiled), rows 64-127 pick dst j (ones row per block)
    S = np.zeros((2 * N, E), f32)
    jj = np.repeat(np.arange(N), N)
    ii = np.tile(np.arange(N), N)
    S[ii, np.arange(E)] = 1.0
    S[N + jj, np.arange(E)] = 1.0
    d["S"] = S
    return d


# constants kept fp32 in SBUF; the rest load as bf16 via gpsimd casting DMA
_F32_CONSTS = {"biases", "ident", "msc", "inv64", "nerep", "ones3", "onesr"}


def _build(n_graphs, num_devices):
    import concourse.bacc as bacc
    import concourse.tile as tile
    import concourse.mybir as mybir
    from contextlib import ExitStack

    dt = mybir.dt
    f32 = dt.float32
    bf16 = dt.float16  # fp16: same PE throughput as bf16, 4x finer mantissa
    Silu = mybir.ActivationFunctionType.Silu
    add = mybir.AluOpType.add
    sub = mybir.AluOpType.subtract
    mult = mybir.AluOpType.mult
    AX = mybir.AxisListType.X

    nc = bacc.Bacc("TRN2", num_devices=num_devices, enable_partition_id=False)

    dr = {}
    for name, shape in [
        ("xin", [n_graphs, N, C]),
        ("xtin", [n_graphs, C, N]),
        ("S", [2 * N, E]),
        ("w1a", [H, L * H]),
        ("w1b", [H, L * H]),
        ("wrow", [1, L * H]),
        ("wm2", [1, L * H]),
        ("w2", [H, L * H]),
        ("cw1", [H, L * H]),
        ("cw2", [H, L]),
        ("nw1a", [H, L * H]),
        ("nw1b", [H, L * H]),
        ("nw2", [H, L * H]),
        ("biases", [H, 5 * L]),
        ("nerep", [H, N]),
        ("ident", [N, N]),
        ("msc", [N, 2]),
        ("inv64", [1, N]),
        ("ones3", [C, 1]),
        ("onesr", [1, N]),
    ]:
        dr[name] = nc.dram_tensor(name, shape, f32, kind="ExternalInput").ap()
    y = nc.dram_tensor("y", [n_graphs, N, C], f32, kind="ExternalOutput").ap()

    with nc.allow_low_precision(reason="bf16 edge MLP"), tile.TileContext(nc) as tc, ExitStack() as es:
        cp = es.enter_context(tc.tile_pool(name="const", bufs=1))
        sp = es.enter_context(tc.tile_pool(name="state", bufs=1))
        wp = es.enter_context(tc.tile_pool(name="work", bufs=2))
        tp = es.enter_context(tc.tile_pool(name="tchunk", bufs=4))
        ep = es.enter_context(tc.tile_pool(name="edge", bufs=3))
        bp = es.enter_context(tc.tile_pool(name="bigp", bufs=2, space="PSUM"))
        sm = es.enter_context(tc.tile_pool(name="smps", bufs=2, space="PSUM"))
        dp = es.enter_context(tc.tile_pool(name="dram", bufs=4, space="DRAM"))

        ct = {}
        for name in (
            "S", "w1a", "w1b", "wrow", "wm2", "w2", "cw1", "cw2",
            "nw1a", "nw1b", "nw2", "biases", "nerep", "ident", "msc", "inv64",
            "ones3", "onesr",
        ):
            shape = list(dr[name].shape)
            if name in _F32_CONSTS:
                t = cp.tile(shape, f32, tag=f"c_{name}")
                nc.sync.dma_start(out=t[:], in_=dr[name])
            else:
                t = cp.tile(shape, bf16, tag=f"c_{name}")
                nc.gpsimd.dma_start(out=t[:], in_=dr[name])
            ct[name] = t

        def wsl(name, l):  # [128,128] weight slice of layer l
            return ct[name][:, l * H : (l + 1) * H]

        def bsl(bi, l):  # bias column [128,1] fp32
            return ct["biases"][:, bi * L + l : bi * L + l + 1]

        # ---- per-graph state ----
        HTs, H16s, Pxs, PTs, dcws = [], [], [], [], []
        for g in range(n_graphs):
            HT = sp.tile([H, N], f32, tag=f"HT{g}")
            nc.sync.dma_start(out=HT[:], in_=dr["nerep"][:, :])
            H16 = sp.tile([H, N], bf16, tag=f"H16{g}")
            nc.gpsimd.dma_start(out=H16[:], in_=dr["nerep"][:, :])
            Px = sp.tile([N, 4], f32, tag=f"Px{g}")
            nc.sync.dma_start(out=Px[:, 0:3], in_=dr["xin"][g])
            nc.vector.memset(Px[:, 3:4], 1.0)
            PT = sp.tile([C, N], f32, tag=f"PT{g}")
            nc.sync.dma_start(out=PT[:], in_=dr["xtin"][g])
            PTm2 = sp.tile([C, N], f32, tag=f"PTm2{g}")
            nc.vector.tensor_scalar_mul(out=PTm2[:], in0=PT[:], scalar1=-2.0)
            dcw = dp.tile([N, N], f32, tag=f"dcw{g}")
            HTs.append(HT)
            H16s.append(H16)
            Pxs.append(Px)
            PTs.append((PT, PTm2))
            dcws.append(dcw)

        def emit_pos_update(g):
            """Apply the deferred coordinate update parked in dcw[g]."""
            Px, (PT, PTm2) = Pxs[g], PTs[g]
            CWM = wp.tile([N, N], f32, tag="CWM")
            nc.sync.dma_start(out=CWM[:], in_=dcws[g][:])
            psT = sm.tile([N, N], f32, tag="sm")
            nc.tensor.transpose(out=psT[:], in_=CWM[:], identity=ct["ident"][:])
            CWT = wp.tile([N, N], f32, tag="CWT")
            nc.vector.tensor_copy(out=CWT[:], in_=psT[:])
            upd = sm.tile([N, 4], f32, tag="sm")
            nc.tensor.matmul(out=upd[:], lhsT=CWT[:], rhs=Px[:], start=True, stop=True)
            upds = wp.tile([N, 4], f32, tag="upds")
            nc.vector.tensor_copy(out=upds[:], in_=upd[:])
            tmp = wp.tile([N, C], f32, tag="tmp")
            nc.vector.tensor_scalar_mul(out=tmp[:], in0=Px[:, 0:3], scalar1=upds[:, 3:4])
            nc.vector.tensor_tensor(out=Px[:, 0:3], in0=Px[:, 0:3], in1=upds[:, 0:3], op=add)
            nc.vector.tensor_tensor(out=Px[:, 0:3], in0=Px[:, 0:3], in1=tmp[:], op=sub)
            ptp = sm.tile([C, N], f32, tag="sm")
            nc.tensor.transpose(out=ptp[:], in_=Px[:, 0:3], identity=ct["ident"][:])
            nc.vector.tensor_copy(out=PT[:], in_=ptp[:])
            nc.vector.tensor_scalar_mul(out=PTm2[:], in0=PT[:], scalar1=-2.0)

        def emit_pre(g, l):
            """D2M = gd(+)gd - 2*gram in fp32 PSUM, one bf16 round; lS = [A'; B']."""
            (PT, PTm2), H16 = PTs[g], H16s[g]
            # gd row: |p_i|^2 as [1,64] via ones3^T @ (PT*PT)
            PTsq = wp.tile([C, N], f32, tag="PTsq")
            nc.vector.tensor_tensor(out=PTsq[:], in0=PT[:], in1=PT[:], op=mult)
            gdp = sm.tile([1, N], f32, tag="sm")
            nc.tensor.matmul(out=gdp[:], lhsT=ct["ones3"][:], rhs=PTsq[:], start=True, stop=True)
            gdr = wp.tile([1, N], f32, tag="gdr")
            nc.vector.tensor_copy(out=gdr[:], in_=gdp[:])
            # D2M[a,b] = -2*gram + gd[a] + gd[b], all in fp32 PSUM
            D2 = sm.tile([N, N], f32, tag="sm")
            nc.tensor.matmul(out=D2[:], lhsT=PTm2[:], rhs=PT[:], start=True, stop=False)
            nc.tensor.matmul(out=D2[:], lhsT=gdr[:], rhs=ct["onesr"][:], start=False, stop=False)
            nc.tensor.matmul(out=D2[:], lhsT=ct["onesr"][:], rhs=gdr[:], start=False, stop=True)
            d2s = wp.tile([N, N], bf16, tag="d2s")
            nc.vector.tensor_copy(out=d2s[:], in_=D2[:])
            dd2 = dp.tile([N, N], bf16, tag="dd2")
            nc.sync.dma_start(out=dd2[:], in_=d2s[:])
            d2row = wp.tile([1, E], bf16, tag="d2row")
            nc.sync.dma_start(out=d2row[:], in_=dd2[:].rearrange("a b -> (a b)")[None, :])

            lS = wp.tile([2 * N, H], bf16, tag="lS")
            for half, wname in ((0, "w1a"), (1, "w1b")):
                ps = sm.tile([N, H], f32, tag="sm")
                nc.tensor.matmul(out=ps[:], lhsT=H16[:], rhs=wsl(wname, l), start=True, stop=True)
                nc.vector.tensor_copy(out=lS[half * N : (half + 1) * N, :], in_=ps[:])
            return lS, d2row

        def emit_edges(g, l, lS, d2row):
            """Stage-batched edge MLP over NCH chunks; parks cw matrix in dcw[g]."""
            m_full = ep.tile([H, E], bf16, tag="m")
            t1s, t2s = [], []
            # stage 1: selection matmul (start) then d2 rank-1 (stop), then silu
            ps1s = []
            for c in range(NCH):
                c0 = c * CH
                ps1 = bp.tile([H, CH], f32, tag="big")
                for q in range(CH // 512):
                    s = slice(c0 + q * 512, c0 + (q + 1) * 512)
                    o = slice(q * 512, (q + 1) * 512)
                    nc.tensor.matmul(out=ps1[:, o], lhsT=lS[:], rhs=ct["S"][:, s], start=True, stop=False)
                for q in range(CH // 512):
                    s = slice(c0 + q * 512, c0 + (q + 1) * 512)
                    o = slice(q * 512, (q + 1) * 512)
                    nc.tensor.matmul(out=ps1[:, o], lhsT=wsl("wrow", l), rhs=d2row[:, s], start=False, stop=True)
                t1 = tp.tile([H, CH], bf16, tag="t1")
                nc.scalar.activation(out=t1[:], in_=ps1[:], func=Silu, bias=bsl(0, l))
                t1s.append(t1)
            # stage 2: m = silu(t1 @ w2 + b2)
            for c in range(NCH):
                c0 = c * CH
                ps2 = bp.tile([H, CH], f32, tag="big")
                for q in range(CH // 512):
                    o = slice(q * 512, (q + 1) * 512)
                    nc.tensor.matmul(out=ps2[:, o], lhsT=wsl("w2", l), rhs=t1s[c][:, o], start=True, stop=True)
                nc.scalar.activation(out=m_full[:, c0 : c0 + CH], in_=ps2[:], func=Silu, bias=bsl(1, l))
            # stage 3: t2 = silu(m @ cw1 + cb1)
            for c in range(NCH):
                c0 = c * CH
                ps3 = bp.tile([H, CH], f32, tag="big")
                for q in range(CH // 512):
                    o = slice(q * 512, (q + 1) * 512)
                    nc.tensor.matmul(out=ps3[:, o], lhsT=wsl("cw1", l),
                                     rhs=m_full[:, c0 + q * 512 : c0 + (q + 1) * 512], start=True, stop=True)
                t2 = tp.tile([H, CH], bf16, tag="t2")
                nc.scalar.activation(out=t2[:], in_=ps3[:], func=Silu, bias=bsl(2, l))
                t2s.append(t2)
            # cw = t2 @ cw2: 4 col-tiled concurrent matmuls -> partitions 0/32/64/96
            for grp in range(2):
                cwp = sm.tile([H, 512], f32, tag="cwp")
                for q in range(4):
                    t2c = t2s[grp * 2 + q // 2]
                    o = slice((q % 2) * 512, (q % 2 + 1) * 512)
                    nc.tensor.matmul(out=cwp[32 * q : 32 * q + 1, :], lhsT=ct["cw2"][:, l : l + 1],
                                     rhs=t2c[:, o], start=True, stop=True, tile_position=(0, 32 * q))
                cwst = wp.tile([97, 512], f32, tag="cwst")
                nc.vector.tensor_copy(out=cwst[:], in_=cwp[0:97, :])
                for q in range(4):
                    o0 = grp * 2048 + q * 512
                    nc.sync.dma_start(
                        out=dcws[g][:].rearrange("a b -> (a b)")[None, o0 : o0 + 512],
                        in_=cwst[32 * q : 32 * q + 1, :],
                    )
            return m_full

        def emit_msg_node(g, l, m_full):
            HT, H16 = HTs[g], H16s[g]
            # diag first (the fold below overwrites it for j<32), then fold
            # i in [32,64) onto [0,32) on GpSimd and DVE inner reduce
            mdiag = wp.tile([H, N], bf16, tag="mdiag")
            nc.vector.tensor_copy(out=mdiag[:], in_=m_full[:, :: N + 1])
            mv = m_full[:].rearrange("p (j i) -> p j i", i=N)
            nc.gpsimd.tensor_tensor(out=mv[:, :, 0:32], in0=mv[:, :, 0:32], in1=mv[:, :, 32:64], op=add)
            msg = wp.tile([H, N], bf16, tag="msg")
            nc.vector.tensor_reduce(out=msg[:], in_=mv[:, :, 0:32], axis=AX, op=add)
            nc.vector.tensor_tensor(out=msg[:], in0=msg[:], in1=mdiag[:], op=sub)

            nps = sm.tile([H, N], f32, tag="sm")
            nc.tensor.matmul(out=nps[:], lhsT=wsl("nw1a", l), rhs=H16[:], start=True, stop=False)
            nc.tensor.matmul(out=nps[:], lhsT=wsl("nw1b", l), rhs=msg[:], start=False, stop=True)
            u = wp.tile([H, N], bf16, tag="u")
            nc.scalar.activation(out=u[:], in_=nps[:], func=Silu, bias=bsl(3, l))
            nps2 = sm.tile([H, N], f32, tag="sm")
            nc.tensor.matmul(out=nps2[:], lhsT=wsl("nw2", l), rhs=u[:], start=True, stop=True)
            nc.vector.tensor_tensor(out=HT[:], in0=HT[:], in1=nps2[:], op=add)
            nc.vector.tensor_scalar_add(out=HT[:], in0=HT[:], scalar1=bsl(4, l))
            if l < L - 1:
                nc.vector.tensor_copy(out=H16[:], in_=HT[:])

        # ---- main schedule: pre-work pipelined one graph ahead ----
        pre = {}
        for l in range(L):
            for g in range(n_graphs):
                if g == 0:
                    if l > 0:
                        emit_pos_update(0)
                    pre[0] = emit_pre(0, l)
                if g + 1 < n_graphs:
                    if l > 0:
                        emit_pos_update(g + 1)
                    pre[g + 1] = emit_pre(g + 1, l)
                lS, d2row = pre.pop(g)
                m_full = emit_edges(g, l, lS, d2row)
                emit_msg_node(g, l, m_full)

        # ---- finalize: last pos update, dx = P - P0, mean-center, scale ----
        for g in range(n_graphs):
            emit_pos_update(g)
            Px = Pxs[g]
            p0 = wp.tile([N, C], f32, tag="p0")
            nc.sync.dma_start(out=p0[:], in_=dr["xin"][g])
            dxt = wp.tile([N, C], f32, tag="dxt")
            nc.vector.tensor_tensor(out=dxt[:], in0=Px[:, 0:3], in1=p0[:], op=sub)
            mean = sm.tile([1, C], f32, tag="sm")
            nc.tensor.matmul(out=mean[:], lhsT=ct["msc"][:, 0:1], rhs=dxt[:], start=True, stop=True)
            means = wp.tile([1, C], f32, tag="means")
            nc.vector.tensor_copy(out=means[:], in_=mean[:])
            mrep = sm.tile([N, C], f32, tag="sm")
            nc.tensor.matmul(out=mrep[:], lhsT=ct["inv64"][:], rhs=means[:], start=True, stop=True)
            nc.vector.tensor_tensor(out=dxt[:], in0=dxt[:], in1=mrep[:], op=sub)
            nc.vector.tensor_scalar_mul(out=dxt[:], in0=dxt[:], scalar1=ct["msc"][:, 1:2])
            nc.sync.dma_start(out=y[g], in_=dxt[:])

    nc.compile()
    return nc


def _get_nc(n_graphs, num_devices):
    key = (n_graphs, num_devices)
    if key not in _CACHE:
        _CACHE[key] = _build(n_graphs, num_devices)
    return _CACHE[key]


def make_in_maps(inputs, n_graphs=GPC, ncores=NCORES):
    consts = _prep_consts(inputs)
    x = np.asarray(inputs["x"], np.float32)
    in_maps = []
    for c in range(ncores):
        xs = x[c * n_graphs : (c + 1) * n_graphs].reshape(n_graphs, N, C)
        m = dict(consts)
        m["xin"] = np.ascontiguousarray(xs)
        m["xtin"] = np.ascontiguousarray(xs.transpose(0, 2, 1))
        in_maps.append(m)
    return in_maps


def kernel(**inputs) -> np.ndarray:
    from concourse.bass_utils import run_bass_kernel_spmd

    nc = _get_nc(GPC, NCORES)
    in_maps = make_in_maps(inputs)
    res = run_bass_kernel_spmd(nc, in_maps, core_ids=list(range(NCORES)), trace=False)
    outs = [res.results[c]["y"].reshape(GPC, N * C) for c in range(NCORES)]
    return np.concatenate(outs, axis=0).astype(np.float32)
